# revision 8
# baseline (speedup 1.0000x reference)
"""VQ codebook bottleneck block (eval forward) on 8 Trainium2 NeuronCores.

Data-parallel: the flattened token dim (N*T = 65536 tokens) is sharded across
8 cores; since T == 2048 and each batch row of x is a contiguous (width, T)
slab, each core takes 4 full batches of x.  The codebook is replicated.

Per-core device program (all fp32 — argmin near-ties require fp32-grade
matmul precision; bf16 flips hundreds of indices):
  score[t, j] = 2*<x_t, k_j> - ||k_j||^2         (PE matmul, K=512 in 4 chunks,
                                                  argmin dist == argmax score)
  max/argmax over 2048 bins                       (DVE tensor_tensor_reduce +
                                                  max_index)
  x_d gather: k[argmax] per token                 (GPSIMD indirect DMA)
  transpose gathered rows to (width, T) layout    (PE transpose via identity)
  partial sums for prenorm/fit/commit_loss        (DVE reductions + PE ones-
                                                  matmul partition reduce)
Host combines per-core partials in fp64.
"""

import numpy as np

_CACHE = {}

# debug scoping: "full" | "noxd" (skip gather/transpose/xd) | "nomax"
# (skip max_index/gather/xd/xl) | "mm" (matmuls+stats only)
SCOPE = "full"

N_CORES = 8
NB = 4            # batches per core (32 / 8)
W = 512           # emb width
T = 2048          # tokens per batch
KB = 2048         # codebook bins
P = 128
WC = W // P       # 4 width chunks (contraction)
TT = T // P       # 16 token tiles per batch
NTOK = NB * T     # tokens per core


def _build_program():
    import concourse.bass as bass
    import concourse.mybir as mybir
    import concourse.tile as tile
    from concourse import bacc
    from concourse.masks import make_identity
    from contextlib import ExitStack

    f32 = mybir.dt.float32
    u32 = mybir.dt.uint32
    Alu = mybir.AluOpType
    Ax = mybir.AxisListType

    nc = bacc.Bacc(
        "TRN2",
        target_bir_lowering=False,
        debug=False,
        num_devices=N_CORES,
    )

    x_d = nc.declare_dram_parameter("x", [NB, W, T], f32, isOutput=False)
    kt2_d = nc.declare_dram_parameter("kt2", [W, KB], f32, isOutput=False)
    nk2_d = nc.declare_dram_parameter("nk2", [P, KB], f32, isOutput=False)
    kcb_d = nc.declare_dram_parameter("kcb", [KB, W], f32, isOutput=False)

    xd_d = nc.declare_dram_parameter("xd", [NB, W, T], f32, isOutput=True)
    xl_d = nc.declare_dram_parameter("xl", [NB, T], u32, isOutput=True)
    st_d = nc.declare_dram_parameter("st", [3, 1], f32, isOutput=True)

    FMIN = -3.4e38

    with tile.TileContext(nc) as tc, ExitStack() as ctx:
        const = ctx.enter_context(tc.tile_pool(name="const", bufs=1))
        xpool = ctx.enter_context(tc.tile_pool(name="xpool", bufs=2))
        spool = ctx.enter_context(tc.tile_pool(name="spool", bufs=2))
        gpool = ctx.enter_context(tc.tile_pool(name="gpool", bufs=3))
        xdpool = ctx.enter_context(tc.tile_pool(name="xdpool", bufs=3))
        small = ctx.enter_context(tc.tile_pool(name="small", bufs=3))
        accp = ctx.enter_context(tc.tile_pool(name="accp", bufs=1))
        scratch = ctx.enter_context(tc.tile_pool(name="scratch", bufs=2))
        pspool = ctx.enter_context(tc.tile_pool(name="pspool", bufs=3, space="PSUM"))
        tpool = ctx.enter_context(tc.tile_pool(name="tpool", bufs=2, space="PSUM"))

        # constants
        kt2_sb = const.tile([P, WC, KB], f32, tag="kt2")
        nc.sync.dma_start(kt2_sb[:], kt2_d.rearrange("(c p) j -> p c j", p=P))
        nk2_sb = const.tile([P, KB], f32, tag="nk2")
        nc.sync.dma_start(nk2_sb[:], nk2_d[:])
        ident = const.tile([P, P], f32, tag="ident")
        make_identity(nc, ident[:])
        ones = const.tile([P, 1], f32, tag="ones")
        nc.vector.memset(ones[:], 1.0)

        # accumulators
        macc = accp.tile([P, NB * TT], f32, tag="macc")
        sxacc = accp.tile([P, NB * WC], f32, tag="sxacc")
        sx2acc = accp.tile([P, NB * WC], f32, tag="sx2acc")

        pending = None  # deferred gather->transpose->store stage (1-tile SW pipeline)

        def flush_pending():
            nonlocal pending
            if pending is None:
                return
            G, idx8, n_, t_ = pending
            tok_ = slice(t_ * P, (t_ + 1) * P)
            tps = tpool.tile([P, W], f32, tag="tp")
            for wb in range(WC):
                nc.tensor.transpose(
                    tps[:, wb * P:(wb + 1) * P], G[:, wb * P:(wb + 1) * P], ident[:]
                )
            xdt = xdpool.tile([P, W], f32, tag="xd")
            nc.scalar.copy(xdt[:], tps[:])
            nc.sync.dma_start(
                xd_d[n_, :, tok_].rearrange("(wb p) t -> p wb t", p=P),
                xdt[:].rearrange("p (wb t) -> p wb t", wb=WC),
            )
            nc.sync.dma_start(xl_d[n_, tok_], idx8[:, 0:1])
            pending = None

        for n in range(NB):
            x_sb = xpool.tile([P, WC, T], f32, tag="x")
            nc.sync.dma_start(x_sb[:], x_d[n].rearrange("(c p) t -> p c t", p=P))

            # element stats (prenorm / fit): sum x and sum x^2 per chunk.
            # Runs on the (otherwise mostly idle) ScalarEngine via
            # activation accum_out (= row-sum of the activated output).
            for c in range(WC):
                ch = n * WC + c
                scr = scratch.tile([P, T], f32, tag="scr")
                nc.scalar.activation(
                    out=scr[:], in_=x_sb[:, c],
                    func=mybir.ActivationFunctionType.Identity,
                    accum_out=sxacc[:, ch:ch + 1],
                )
                scr2 = scratch.tile([P, T], f32, tag="scr")
                nc.scalar.activation(
                    out=scr2[:], in_=x_sb[:, c],
                    func=mybir.ActivationFunctionType.Square,
                    accum_out=sx2acc[:, ch:ch + 1],
                )

            for t in range(TT):
                ti = n * TT + t
                tok = slice(t * P, (t + 1) * P)
                score_sb = spool.tile([P, KB], f32, tag="score")
                for h in range(2):
                    ps = pspool.tile([P, 1024], f32, tag="ps")
                    for b2 in range(2):
                        b = h * 2 + b2
                        for kc in range(WC):
                            nc.tensor.matmul(
                                ps[:, b2 * 512:(b2 + 1) * 512],
                                lhsT=x_sb[:, kc, tok],
                                rhs=kt2_sb[:, kc, b * 512:(b + 1) * 512],
                                start=(kc == 0),
                                stop=(kc == WC - 1),
                            )
                    # bias add (-||k||^2) while moving PSUM -> SBUF
                    nc.vector.tensor_tensor(
                        out=score_sb[:, h * 1024:(h + 1) * 1024],
                        in0=ps[:],
                        in1=nk2_sb[:, h * 1024:(h + 1) * 1024],
                        op=Alu.add,
                    )
                mx8 = small.tile([P, 8], f32, tag="mx8")
                nc.vector.max(out=mx8[:], in_=score_sb[:])
                # per-token max score into accumulator column (for fit)
                nc.vector.tensor_copy(macc[:, ti:ti + 1], mx8[:, 0:1])
                if SCOPE in ("nomax", "mm"):
                    continue
                idx8 = small.tile([P, 8], u32, tag="idx8")
                nc.vector.max_index(idx8[:], mx8[:], score_sb[:])

                if SCOPE == "noxd":
                    nc.sync.dma_start(xl_d[n, tok], idx8[:, 0:1])
                    continue

                # gather codebook rows for these 128 tokens
                G = gpool.tile([P, W], f32, tag="g")
                nc.gpsimd.indirect_dma_start(
                    out=G[:],
                    out_offset=None,
                    in_=kcb_d[:],
                    in_offset=bass.IndirectOffsetOnAxis(ap=idx8[:, 0:1], axis=0),
                )

                flush_pending()
                pending = (G, idx8, n, t)

        flush_pending()

        # final scalars: partition-reduce [sum_x, sum_x2, sum_maxscore] via matmul
        red3 = accp.tile([P, 3], f32, tag="red3")
        nc.vector.tensor_reduce(out=red3[:, 0:1], in_=sxacc[:], axis=Ax.X, op=Alu.add)
        nc.vector.tensor_reduce(out=red3[:, 1:2], in_=sx2acc[:], axis=Ax.X, op=Alu.add)
        nc.vector.tensor_reduce(out=red3[:, 2:3], in_=macc[:], axis=Ax.X, op=Alu.add)
        ps3 = tpool.tile([P, 1], f32, tag="tp")
        nc.tensor.matmul(ps3[:3, 0:1], lhsT=red3[:], rhs=ones[:], start=True, stop=True)
        st_sb = small.tile([P, 1], f32, tag="st")
        nc.scalar.copy(st_sb[:3], ps3[:3, 0:1])
        nc.sync.dma_start(st_d[:], st_sb[:3, :])

    nc.compile()
    return nc


def _get_program():
    if "nc" not in _CACHE:
        _CACHE["nc"] = _build_program()
    return _CACHE["nc"]


def kernel(x, k, _want_profile=False):
    from concourse.bass_utils import run_bass_kernel_spmd

    x = np.asarray(x)
    k = np.asarray(k)
    assert x.shape == (32, 512, 2048) and k.shape == (2048, 512)

    xf32 = np.ascontiguousarray(x, dtype=np.float32)
    kf32 = np.ascontiguousarray(k, dtype=np.float32)

    kt2 = np.ascontiguousarray((2.0 * kf32).T)                 # [512, 2048]
    nk2_row = -np.sum(kf32 * kf32, axis=-1)                    # [2048] fp32
    nk2 = np.ascontiguousarray(np.broadcast_to(nk2_row[None, :], (P, KB)))

    nc = _get_program()

    in_maps = []
    for c in range(N_CORES):
        in_maps.append({
            "x": np.ascontiguousarray(xf32[c * NB:(c + 1) * NB]),
            "kt2": kt2,
            "nk2": nk2,
            "kcb": kf32,
        })

    res = run_bass_kernel_spmd(nc, in_maps, core_ids=list(range(N_CORES)),
                               trace=_want_profile)

    xd = np.concatenate([np.asarray(r["xd"]) for r in res.results], axis=0)
    xl = np.concatenate([np.asarray(r["xl"]) for r in res.results], axis=0)
    xl = xl.astype(np.int32)

    sx = 0.0
    sx2 = 0.0
    smax = 0.0
    for r in res.results:
        st = np.asarray(r["st"], dtype=np.float64).reshape(-1)
        sx += st[0]
        sx2 += st[1]
        smax += st[2]

    n_elem = float(32 * T * W)
    n_tok = float(32 * T)
    prenorm = np.float32(np.sqrt(max(sx2 - sx * sx / n_elem, 0.0) / n_elem))
    fit = np.float32((sx2 - smax) / n_tok)
    commit_loss = np.float32((sx2 - smax) / n_elem)

    if _want_profile:
        _CACHE["last_exec_time_ns"] = res.exec_time_ns
        _CACHE["last_mean_exec_time_ns"] = res.mean_exec_time_ns

    return (
        xl,
        xd,
        np.array(commit_loss, dtype=np.float32),
        np.array(fit, dtype=np.float32),
        np.array(prenorm, dtype=np.float32),
    )
